# revision 3
# baseline (speedup 1.0000x reference)
"""Trainium2 Bass kernel for nn_Attention (B=4, N=2048, D=1024, H=8 heads).

Computes: qkv = x @ Wkv.T; q,k,v split into 8 heads of 128 dims;
y = softmax(q k^T / sqrt(128) + mask) v;  out = y @ Wo.T + bo.

Sharding (8 NeuronCores): core (b, g) = batch b in 0..3, head-group g in 0..1
(4 heads each).  Each core computes its 4 heads' attention and a partial
output projection; the host sums the two head-group partials per batch and
adds bo.

The additive mask is skipped on device: the problem spec fills it with zeros
(exp(s + 0) == exp(s)).  If a nonzero mask is ever passed, kernel() falls back
to an exact numpy implementation.

Device-side design (v3):
 - All matmul operands are 16-bit (full PE rate): x/Wkv in bf16, q/k/v/
   exp-tiles/Wo in fp16.  Accumulation stays fp32 in PSUM.
 - The host pre-transposes AND pre-casts x, Wkv and Wo slices per core,
   so the device does NO transposes and no casts.  wkvt is laid out
   [q0 k0 | q1 k1 | q2 k2 | q3 k3 | v(all 4 heads, 512)] so that
   (a) q/k project per head into the transposed [d, n] layout the
   scores matmul wants, and (b) v projects DIRECTLY into its natural
   [n, d] layout (lhsT = x tile, rhs = the 512-wide v weight block),
   which removes all 64 PE-transposes and their DVE drains from v2.
 - Scores are computed TRANSPOSED (sT[k, q] = kT_tile.T @ qT) so softmax
   needs no p transposes before the PV matmul.
 - exp runs on the scalar engine reading scores from PSUM with the
   1/sqrt(128) scale fused in, writing fp16 tiles to SBUF.
 - The softmax denominator is a running fp16 elementwise sum of the 16
   exp tiles on DVE, finished by a single ones-matmul for the
   cross-partition reduction.
 - Emission: prologue = head-0 q/k projection + ALL v n-tiles (overlaps
   the input DMA; the Tile scheduler reorders by data arrival).  Heads
   1-3's q/k projections and the first half of the output projection are
   interleaved into the attention blocks as single-matmul fillers so the
   PE never drains while the scalar engine works through the exp stream.
 - PSUM budget (8 banks): scores "st" 2x[128,1024]f32 (4; also time-shares
   with the v-projection accumulator and the den ones-matmul), yacc "acc"
   (2; time-shares with head-0 k cells), proj/oproj "pp" (2).
"""

import numpy as np

B, N, D, H = 4, 2048, 1024, 8
HD = D // H          # 128 head dim
HPC = H // 2         # 4 heads per core
DY = HPC * HD        # 512 local y dims per core
P = 128
NT = N // P          # 16 n-tiles
DC = D // P          # 8 d-chunks
KT = N // P          # 16 k-tiles
QC = 2               # q chunks per head
QW = N // QC         # 1024 q width
MM = 512             # max fp32 moving free dim
NS = 4               # x load n-slices
SW = N // NS         # 512 slice width
QKW = HPC * 2 * HD   # 1024 q/k columns in wkvt
SCALE = float(1.0 / np.sqrt(HD))

_CACHE = {}


def _build():
    from contextlib import ExitStack

    import concourse.bacc as bacc
    import concourse.bass as bass
    import concourse.mybir as mybir
    from concourse.tile import TileContext

    ts = bass.ts
    F32 = mybir.dt.float32
    F16 = mybir.dt.float16
    BF16 = mybir.dt.bfloat16
    EXP = mybir.ActivationFunctionType.Exp

    nc = bacc.Bacc("TRN2", target_bir_lowering=False, debug=False)
    # Host-pre-transposed, host-pre-cast inputs (see make_in_maps):
    #   xt[d, n]                    = x[b][n, d]                      (bf16)
    #   wkvt[d, h*256 + 128c + i]   = W{q,k}[g*DY + h*HD + i, d]      (bf16)
    #   wkvt[d, 1024 + j]           = Wv[g*DY + j, d]                 (bf16)
    #   wot[p, h*D + e]             = Wo[e, g*DY + h*HD + p]          (fp16)
    xt = nc.dram_tensor("xt", [D, N], BF16, kind="ExternalInput")
    wkvt = nc.dram_tensor("wkvt", [D, 3 * DY], BF16, kind="ExternalInput")
    wot = nc.dram_tensor("wot", [DY, D], F16, kind="ExternalInput")
    # fp16 output halves the output DMA; the host sums the two head-group
    # partials in fp32.
    out = nc.dram_tensor("out", [N, D], F16, kind="ExternalOutput")

    with TileContext(nc) as tc, ExitStack() as top:
        consts = top.enter_context(tc.tile_pool(name="consts", bufs=1))
        ones32 = consts.tile([P, P], F32, tag="ones32")
        nc.vector.memset(ones32, 1.0)
        ones16 = consts.tile([P, P], F16, tag="ones16")
        nc.vector.tensor_copy(out=ones16, in_=ones32)

        persist = top.enter_context(tc.tile_pool(name="persist", bufs=1))
        xTf = persist.tile([P, DC, N], BF16, tag="xTf")
        wkvTf = persist.tile([P, DC, 3 * DY], BF16, tag="wkvTf")
        woTf = persist.tile([P, HPC, D], F16, tag="woTf")
        # qT/kT are double-buffered on head parity so head h+1's projection
        # (interleaved into head h's attention) never overwrites tiles
        # attention is still reading.  v (vna) is shared by all heads and
        # computed once in the prologue, so it needs no parity buffer.
        qT = [persist.tile([P, N], F16, tag=f"qT{i}", name=f"qT{i}") for i in range(2)]
        kT = [persist.tile([P, N], F16, tag=f"kT{i}", name=f"kT{i}") for i in range(2)]
        vna = persist.tile([P, NT, DY], F16, tag="vna")
        yT = persist.tile([P, HPC, N], F16, tag="yT")

        work = top.enter_context(tc.tile_pool(name="work", bufs=1))
        psum = top.enter_context(tc.tile_pool(name="ps", bufs=1, space="PSUM"))

        # ---------------- loaders (direct DMA into SBUF tiles) ----------------
        def load_x_slice(ns):
            for j in range(DC):
                nc.sync.dma_start(
                    out=xTf[:, j, ts(ns, SW)], in_=xt.ap()[ts(j, P), ts(ns, SW)]
                )

        def load_wkv_qk(h):
            for j in range(DC):
                nc.sync.dma_start(
                    out=wkvTf[:, j, ts(h, 2 * HD)],
                    in_=wkvt.ap()[ts(j, P), ts(h, 2 * HD)],
                )

        def load_wkv_v():
            for j in range(DC):
                nc.sync.dma_start(
                    out=wkvTf[:, j, QKW : QKW + DY],
                    in_=wkvt.ap()[ts(j, P), QKW : QKW + DY],
                )

        def load_wo():
            for h in range(HPC):
                nc.sync.dma_start(out=woTf[:, h, :], in_=wot.ap()[ts(h, P), :])

        # ------------- q/k projection thunks for head h -------------
        # Each thunk emits one PE instruction (plus the trailing DVE drain),
        # so attention blocks can interleave them as fillers.  A cell is one
        # (c, half) pair: a [128, 1024] PSUM accumulation over the 8 d-chunks
        # x 2 n-halves, drained to qT/kT once complete.
        def proj_thunks(h, cell_tags):
            hb = h % 2
            cells = {}
            tag_of = {}
            for idx, key in enumerate((c, half) for half in range(2) for c in range(2)):
                tag_of[key] = cell_tags[idx % len(cell_tags)]
            dests = {0: qT[hb], 1: kT[hb]}

            def mk(c, half, nch, jj):
                def emit():
                    key = (c, half)
                    if key not in cells:
                        cells[key] = psum.tile(
                            [P, QW], F32, tag=tag_of[key], bufs=_BUFS[tag_of[key]],
                            name=f"pp{h}{c}{half}",
                        )
                    pp = cells[key]
                    col0 = h * 2 * HD + c * HD
                    nc.tensor.matmul(
                        pp[:, ts(nch, MM)],
                        wkvTf[:, jj, col0 : col0 + HD],
                        xTf[:, jj, half * QW + nch * MM : half * QW + (nch + 1) * MM],
                        start=(jj == 0),
                        stop=(jj == DC - 1),
                    )
                    if jj == DC - 1 and nch == 1:
                        nc.vector.tensor_copy(out=dests[c][:, ts(half, QW)], in_=pp)
                return emit

            return [
                mk(c, half, nch, jj)
                for half in range(2)
                for c in range(2)
                for nch in range(2)
                for jj in range(DC)
            ]

        _BUFS = {"pp": 1, "st": 2, "acc": 1}

        # ------------- v projection thunks (natural layout, all heads) -------
        def vna_thunks(k):
            cell = {}

            def mk(jj):
                def emit():
                    if "ps" not in cell:
                        cell["ps"] = psum.tile(
                            [P, DY], F32, tag="st", bufs=2, name=f"vps{k}"
                        )
                    ps = cell["ps"]
                    nc.tensor.matmul(
                        ps,
                        xTf[:, jj, ts(k, P)],
                        wkvTf[:, jj, QKW : QKW + DY],
                        start=(jj == 0),
                        stop=(jj == DC - 1),
                    )
                    if jj == DC - 1:
                        nc.scalar.copy(out=vna[:, k, :], in_=ps)
                return emit

            return [mk(jj) for jj in range(DC)]

        # ------------- output projection thunks (n-tile i) -------------
        def oproj_thunks(i, tag):
            cell = {}

            def mk(eh, hh):
                def emit():
                    if "po" not in cell:
                        cell["po"] = psum.tile(
                            [P, D], F32, tag=tag, bufs=_BUFS[tag], name=f"po{i}"
                        )
                    po = cell["po"]
                    nc.tensor.matmul(
                        po[:, ts(eh, MM)],
                        yT[:, hh, ts(i, P)],
                        woTf[:, hh, eh * MM : (eh + 1) * MM],
                        start=(hh == 0),
                        stop=(hh == HPC - 1),
                    )
                    if eh == 1 and hh == HPC - 1:
                        ot = work.tile([P, D], F16, tag="so", bufs=3, name=f"ot{i}")
                        nc.scalar.copy(out=ot, in_=po)
                        nc.sync.dma_start(out=out.ap()[ts(i, P), :], in_=ot)
                return emit

            return [mk(eh, hh) for eh in range(2) for hh in range(HPC)]

        # ------------- attention block for (head h, q-chunk qc) -------------
        def attention(h, qc, fillers, nfill, deferred=None):
            """One attention block.  Returns a 'finisher' closure (den
            cross-partition reduce + normalize) that the CALLER emits inside
            the NEXT block (at k==1) -- emitting it here would head-of-line
            block the next block's scores behind the final DVE den-add."""
            hb = h % 2
            yacc = psum.tile([P, QW], F32, tag="acc", bufs=1, name=f"yacc{h}{qc}")
            dacc = None
            ets = []
            for k in range(KT):
                st = psum.tile([P, QW], F32, tag="st", bufs=2, name=f"st{h}{qc}{k}")
                for m in range(2):
                    nc.tensor.matmul(
                        st[:, ts(m, MM)],
                        kT[hb][:, ts(k, P)],
                        qT[hb][:, qc * QW + m * MM : qc * QW + (m + 1) * MM],
                        start=True,
                        stop=True,
                    )
                et = work.tile([P, QW], F16, tag="et", bufs=5, name=f"et{h}{qc}{k}")
                nc.scalar.activation(out=et, in_=st, func=EXP, scale=SCALE)
                if k == 1 and deferred is not None:
                    deferred()
                # fillers run while the scalar engine works through exp
                for _ in range(nfill):
                    if fillers:
                        fillers.popleft()()
                for m in range(2):
                    nc.tensor.matmul(
                        yacc[:, ts(m, MM)],
                        vna[:, k, ts(h, HD)],
                        et[:, ts(m, MM)],
                        start=(k == 0),
                        stop=(k == KT - 1),
                    )
                # denominator: running fp16 sum of exp tiles on DVE
                if k == 0:
                    ets.append(et)
                elif k == 1:
                    dacc = work.tile([P, QW], F16, tag="dacc", bufs=2, name=f"da{h}{qc}{k}")
                    nc.vector.tensor_add(out=dacc, in0=ets[0], in1=et)
                else:
                    nd = work.tile([P, QW], F16, tag="dacc", bufs=2, name=f"da{h}{qc}{k}")
                    nc.vector.tensor_add(out=nd, in0=dacc, in1=et)
                    dacc = nd
            # Drain yacc now (scalar engine) so the next block's PV can
            # reuse the accumulation bank promptly.
            ysb = work.tile([P, QW], F32, tag="ysb", bufs=2, name=f"ysb{h}{qc}")
            nc.scalar.copy(out=ysb, in_=yacc)
            dacc_f = dacc

            def finisher():
                dmm = psum.tile([P, QW], F32, tag="st", bufs=2, name=f"dmm{h}{qc}")
                for m in range(2):
                    nc.tensor.matmul(
                        dmm[:, ts(m, MM)], ones16, dacc_f[:, ts(m, MM)],
                        start=True, stop=True,
                    )
                rc = work.tile([P, QW], F32, tag="rc", bufs=2, name=f"rc{h}{qc}")
                nc.vector.reciprocal_approx_fast(out=rc, in_=dmm)
                nc.vector.tensor_mul(out=yT[:, h, ts(qc, QW)], in0=ysb, in1=rc)

            return finisher

        # ---------------- emission schedule ----------------
        from collections import deque

        # interleave the first wkv qk-chunk DMAs with x slice 0 so the PE's
        # first matmul starts as early as possible; the v block follows (it
        # is first consumed at prologue position ~32, after the half-0 q/k
        # cells) and x slice 1 after that.
        for j in range(DC):
            nc.sync.dma_start(
                out=wkvTf[:, j, 0 : 2 * HD], in_=wkvt.ap()[ts(j, P), 0 : 2 * HD]
            )
            nc.sync.dma_start(out=xTf[:, j, 0:SW], in_=xt.ap()[ts(j, P), 0:SW])
        load_wkv_v()
        load_x_slice(1)

        # Prologue: head-0 q/k projection + ALL v n-tiles, in x-slice order.
        # The Tile scheduler reorders by data arrival; emission order only
        # sets priority.  DMA issues for the later slices and weights are
        # interleaved at fixed positions.
        h0 = proj_thunks(0, cell_tags=("pp", "acc"))
        # h0 layout: [q(half0) 16 | k(half0) 16 | q(half1) 16 | k(half1) 16]
        prologue = (
            h0[0:16] + h0[16:32]
            + [t for k in range(0, 8) for t in vna_thunks(k)]
            + h0[32:48] + h0[48:64]
            + [t for k in range(8, 16) for t in vna_thunks(k)]
        )
        emitted = 0
        for t in prologue:
            if emitted == 8:
                load_x_slice(2)
            elif emitted == 24:
                load_x_slice(3)
            elif emitted == 40:
                load_wkv_qk(1)
            elif emitted == 56:
                load_wo()
            t()
            emitted += 1

        fin = None
        fillers = deque()
        for h in range(HPC):
            if h + 1 < HPC:
                fillers.extend(proj_thunks(h + 1, cell_tags=("pp",)))
                if h + 2 < HPC:
                    load_wkv_qk(h + 2)
                fin = attention(h, 0, fillers, nfill=2, deferred=fin)
                fin = attention(h, 1, fillers, nfill=2, deferred=fin)
                while fillers:
                    fillers.popleft()()
            else:
                # last head: the first half of the output projection fills
                # qc1 (it needs every head's qc0 yT, complete once qc0's
                # finisher has run); the rest follows in the tail.
                fin = attention(h, 0, fillers, nfill=1, deferred=fin)
                op = deque()
                for i in range(NT // 2):
                    op.extend(oproj_thunks(i, tag="pp"))
                fin = attention(h, 1, op, nfill=3, deferred=fin)
                fin()
                while op:
                    op.popleft()()
                for i in range(NT // 2, NT):
                    for t in oproj_thunks(i, tag="st" if i % 2 else "pp"):
                        t()
    nc.finalize()
    return nc


def _get_nc():
    if "nc" not in _CACHE:
        _CACHE["nc"] = _build()
    return _CACHE["nc"]


def make_in_maps(x, Wkv, Wo):
    """Per-core input dicts for core = 2*b + g (host pre-transposes + casts)."""
    from ml_dtypes import bfloat16

    xts = [np.ascontiguousarray(x[b].T).astype(bfloat16) for b in range(B)]
    wkvts, wots = [], []
    for g in range(2):
        rows = np.concatenate(
            [
                Wkv[c * D + g * DY + h * HD : c * D + g * DY + (h + 1) * HD]
                for h in range(HPC)
                for c in range(2)
            ]
            + [Wkv[2 * D + g * DY : 2 * D + (g + 1) * DY]],
            axis=0,
        )  # [1536, 1024] rows: [q0 k0 q1 k1 q2 k2 q3 k3 | v(512)]
        wkvts.append(np.ascontiguousarray(rows.T).astype(bfloat16))
        wots.append(
            np.ascontiguousarray(Wo[:, g * DY : (g + 1) * DY].T).astype(np.float16)
        )
    in_maps = []
    for core in range(8):
        b, g = divmod(core, 2)
        in_maps.append({"xt": xts[b], "wkvt": wkvts[g], "wot": wots[g]})
    return in_maps


def gather_out(results, bo):
    out = np.empty((B, N, D), np.float32)
    for b in range(B):
        out[b] = np.asarray(results[2 * b]["out"], np.float32) + np.asarray(
            results[2 * b + 1]["out"], np.float32
        )
    out += bo.astype(np.float32)
    return out


def _numpy_reference(x, mask, Wkv, Wo, bo):
    """Exact fallback (used only if a nonzero additive mask is passed)."""
    x64 = x.astype(np.float64)
    qkv = x64 @ Wkv.T.astype(np.float64)
    q, k, v = np.split(qkv, 3, axis=-1)
    q = q.reshape(B, N, H, HD).transpose(0, 2, 1, 3)
    k = k.reshape(B, N, H, HD).transpose(0, 2, 1, 3)
    v = v.reshape(B, N, H, HD).transpose(0, 2, 1, 3)
    s = q @ k.transpose(0, 1, 3, 2) / np.sqrt(HD) + mask.astype(np.float64)
    s = s - s.max(axis=-1, keepdims=True)
    p = np.exp(s)
    p /= p.sum(axis=-1, keepdims=True)
    y = (p @ v).transpose(0, 2, 1, 3).reshape(B, N, D)
    return (y @ Wo.T.astype(np.float64) + bo.astype(np.float64)).astype(np.float32)


def kernel(x, mask, Wkv, Wo, bo):
    x = np.asarray(x, dtype=np.float32)
    mask = np.asarray(mask, dtype=np.float32)
    Wkv = np.asarray(Wkv, dtype=np.float32)
    Wo = np.asarray(Wo, dtype=np.float32)
    bo = np.asarray(bo, dtype=np.float32)
    if mask.size and np.abs(mask).max() != 0.0:
        return _numpy_reference(x, mask, Wkv, Wo, bo)

    from concourse.bass_utils import run_bass_kernel_spmd

    nc = _get_nc()
    res = run_bass_kernel_spmd(nc, make_in_maps(x, Wkv, Wo), core_ids=list(range(8)))
    return gather_out(res.results, bo)


if __name__ == "__main__":
    rng = np.random.default_rng(0)
    x = rng.standard_normal((B, N, D), dtype=np.float32)
    mask = np.zeros((N, N), np.float32)
    Wkv = (rng.standard_normal((3 * D, D), dtype=np.float32) / np.sqrt(D)).astype(np.float32)
    Wo = (rng.standard_normal((D, D), dtype=np.float32) / np.sqrt(D)).astype(np.float32)
    bo = np.zeros((D,), np.float32)
    got = kernel(x, mask, Wkv, Wo, bo)
    want = _numpy_reference(x, mask, Wkv, Wo, bo)
    err = np.linalg.norm(got - want) / np.linalg.norm(want)
    print("rel err:", err)


# revision 24
# speedup vs baseline: 1.3869x; 1.3869x over previous
"""Trainium2 Bass kernel for nn_Attention (B=4, N=2048, D=1024, H=8 heads).

Computes: qkv = x @ Wkv.T; q,k,v split into 8 heads of 128 dims;
y = softmax(q k^T / sqrt(128) + mask) v;  out = y @ Wo.T + bo.

Sharding (8 NeuronCores): core (b, g) = batch b in 0..3, head-group g in 0..1
(4 heads each).  Each core computes its 4 heads' attention and a partial
output projection; the host sums the two head-group partials per batch and
adds bo.

The additive mask is skipped on device: the problem spec fills it with zeros
(exp(s + 0) == exp(s)).  If a nonzero mask is ever passed, kernel() falls back
to an exact numpy implementation.

Device-side design (v3):
 - All matmul operands are 16-bit (full PE rate): x/Wkv in bf16, q/k/v/
   exp-tiles/Wo in fp16.  Accumulation stays fp32 in PSUM.
 - The host pre-transposes AND pre-casts x, Wkv and Wo slices per core,
   so the device does NO transposes and no casts.  wkvt is laid out
   [q0 k0 | q1 k1 | q2 k2 | q3 k3 | v(all 4 heads, 512)] so that
   (a) q/k project per head into the transposed [d, n] layout the
   scores matmul wants, and (b) v projects DIRECTLY into its natural
   [n, d] layout (lhsT = x tile, rhs = the 512-wide v weight block),
   which removes all 64 PE-transposes and their DVE drains from v2.
 - Scores are computed TRANSPOSED (sT[k, q] = kT_tile.T @ qT) so softmax
   needs no p transposes before the PV matmul.
 - exp runs on the scalar engine reading scores from PSUM with the
   1/sqrt(128) scale fused in, writing fp16 tiles to SBUF.
 - The softmax denominator is a running fp16 elementwise sum of the 16
   exp tiles on DVE, finished by a single ones-matmul for the
   cross-partition reduction.
 - Emission: prologue = head-0 q/k projection + ALL v n-tiles (overlaps
   the input DMA; the Tile scheduler reorders by data arrival).  Heads
   1-3's q/k projections and the first half of the output projection are
   interleaved into the attention blocks as single-matmul fillers so the
   PE never drains while the scalar engine works through the exp stream.
 - PSUM budget (8 banks): scores "st" 2x[128,1024]f32 (4; also time-shares
   with the v-projection accumulator and the den ones-matmul), yacc "acc"
   (2; time-shares with head-0 k cells), proj/oproj "pp" (2).
"""

import numpy as np

B, N, D, H = 4, 2048, 1024, 8
HD = D // H          # 128 head dim
HPC = H // 2         # 4 heads per core
DY = HPC * HD        # 512 local y dims per core
P = 128
NT = N // P          # 16 n-tiles
DC = D // P          # 8 d-chunks
KT = N // P          # 16 k-tiles
QC = 2               # q chunks per head
QW = N // QC         # 1024 q width
MM = 512             # max fp32 moving free dim
NS = 4               # x load n-slices
SW = N // NS         # 512 slice width
QKW = HPC * 2 * HD   # 1024 q/k columns in wkvt
SCALE = float(1.0 / np.sqrt(HD))

# Schedule knobs (resolved at _build time; the shipped defaults are the
# sim-tuned winners).
FLAGS = dict(
    kick_step=4,      # j-granularity of the kickoff qk/x interleave
    v_first=False,    # v-block DMA before x slice 1
    drain_split=True,   # yacc drain split ACT/DVE vs single scalar copy
    drain_defer=True,   # defer yacc drain into the next block (at k==0)
    fin_split=False,  # half-split finisher (dmm/recip/mul per 512)
    h3_late=True,     # head-3 half-1 projection inside block (3,0)
    po_split=False,   # oproj drain split ACT/DVE
    tail4=True,       # tail oproj over 4 psum slots instead of 2
    nf0=2, nf1=1, nf2=1, nf30=3, nf31=5,  # per-block filler counts
    h0_jzip=True,      # j-interleave head-0 q/k cells in the prologue
    fin_split_last=True,   # half-split only the final finisher
    po_split_tail=False,   # split drains for the last four oproj tiles
    drain_dve=False,  # yacc drain entirely on DVE
    drain_at=0,       # k at which the previous block's yacc drain is emitted
    fin_at=1,         # k at which the previous block's finisher is emitted
)

_CACHE = {}


def _build():
    from contextlib import ExitStack

    import concourse.bacc as bacc
    import concourse.bass as bass
    import concourse.mybir as mybir
    from concourse.tile import TileContext

    ts = bass.ts
    F32 = mybir.dt.float32
    F16 = mybir.dt.float16
    BF16 = mybir.dt.bfloat16
    EXP = mybir.ActivationFunctionType.Exp

    nc = bacc.Bacc("TRN2", target_bir_lowering=False, debug=False)
    # Host-pre-transposed, host-pre-cast inputs (see make_in_maps):
    #   xt[d, n]                    = x[b][n, d]                      (bf16)
    #   wkvt[d, h*256 + 128c + i]   = W{q,k}[g*DY + h*HD + i, d]      (bf16)
    #   wkvt[d, 1024 + j]           = Wv[g*DY + j, d]                 (bf16)
    #   wot[p, h*D + e]             = Wo[e, g*DY + h*HD + p]          (fp16)
    xt = nc.dram_tensor("xt", [D, N], BF16, kind="ExternalInput")
    wkvt = nc.dram_tensor("wkvt", [D, 3 * DY], BF16, kind="ExternalInput")
    wot = nc.dram_tensor("wot", [DY, D], F16, kind="ExternalInput")
    # fp16 output halves the output DMA; the host sums the two head-group
    # partials in fp32.
    out = nc.dram_tensor("out", [N, D], F16, kind="ExternalOutput")

    with TileContext(nc) as tc, ExitStack() as top:
        consts = top.enter_context(tc.tile_pool(name="consts", bufs=1))
        ones32 = consts.tile([P, P], F32, tag="ones32")
        nc.vector.memset(ones32, 1.0)
        ones16 = consts.tile([P, P], F16, tag="ones16")
        nc.vector.tensor_copy(out=ones16, in_=ones32)

        persist = top.enter_context(tc.tile_pool(name="persist", bufs=1))
        xTf = persist.tile([P, DC, N], BF16, tag="xTf")
        wkvTf = persist.tile([P, DC, 3 * DY], BF16, tag="wkvTf")
        woTf = persist.tile([P, HPC, D], F16, tag="woTf")
        # qT/kT are double-buffered on head parity so head h+1's projection
        # (interleaved into head h's attention) never overwrites tiles
        # attention is still reading.  v (vna) is shared by all heads and
        # computed once in the prologue, so it needs no parity buffer.
        qT = [persist.tile([P, N], F16, tag=f"qT{i}", name=f"qT{i}") for i in range(2)]
        kT = [persist.tile([P, N], F16, tag=f"kT{i}", name=f"kT{i}") for i in range(2)]
        vna = persist.tile([P, NT, DY], F16, tag="vna")
        yT = persist.tile([P, HPC, N], F16, tag="yT")

        work = top.enter_context(tc.tile_pool(name="work", bufs=1))
        psum = top.enter_context(tc.tile_pool(name="ps", bufs=1, space="PSUM"))

        # -------- loaders (batched DMA: one instruction per logical block;
        # each dma_start costs ~500ns of SP issue + a completion semaphore,
        # so fewer/bigger transfers start the pipeline much sooner) --------
        def load_x_slice(ns, j0=0, j1=DC):
            nc.sync.dma_start(
                out=xTf[:, j0:j1, ts(ns, SW)],
                in_=xt.ap()[j0 * P : j1 * P, ts(ns, SW)].rearrange(
                    "(j p) n -> p j n", p=P
                ),
            )

        def load_wkv_qk(h, j0=0, j1=DC):
            nc.sync.dma_start(
                out=wkvTf[:, j0:j1, ts(h, 2 * HD)],
                in_=wkvt.ap()[j0 * P : j1 * P, ts(h, 2 * HD)].rearrange(
                    "(j p) c -> p j c", p=P
                ),
            )

        def load_wkv_v():
            nc.sync.dma_start(
                out=wkvTf[:, :, QKW : QKW + DY],
                in_=wkvt.ap()[:, QKW : QKW + DY].rearrange("(j p) c -> p j c", p=P),
            )

        def load_wo():
            nc.sync.dma_start(
                out=woTf,
                in_=wot.ap().rearrange("(h p) e -> p h e", p=P),
            )

        # ------------- q/k projection thunks for head h -------------
        # Each thunk emits one PE instruction (plus the trailing DVE drain),
        # so attention blocks can interleave them as fillers.  A cell is one
        # (c, half) pair: a [128, 1024] PSUM accumulation over the 8 d-chunks
        # x 2 n-halves, drained to qT/kT once complete.
        def proj_thunks(h, cell_tags):
            hb = h % 2
            cells = {}
            tag_of = {}
            for idx, key in enumerate((c, half) for half in range(2) for c in range(2)):
                tag_of[key] = cell_tags[idx % len(cell_tags)]
            dests = {0: qT[hb], 1: kT[hb]}

            def mk(c, half, nch, jj):
                def emit():
                    key = (c, half)
                    if key not in cells:
                        cells[key] = psum.tile(
                            [P, QW], F32, tag=tag_of[key], bufs=_BUFS[tag_of[key]],
                            name=f"pp{h}{c}{half}",
                        )
                    pp = cells[key]
                    col0 = h * 2 * HD + c * HD
                    nc.tensor.matmul(
                        pp[:, ts(nch, MM)],
                        wkvTf[:, jj, col0 : col0 + HD],
                        xTf[:, jj, half * QW + nch * MM : half * QW + (nch + 1) * MM],
                        start=(jj == 0),
                        stop=(jj == DC - 1),
                    )
                    if jj == DC - 1 and nch == 1:
                        nc.vector.tensor_copy(out=dests[c][:, ts(half, QW)], in_=pp)
                return emit

            if FLAGS["h0_jzip"] and cell_tags != ("pp",):
                # head-0 prologue: j-interleave the q and k cells so every
                # arriving x/wkv j-chunk immediately unlocks the next two
                # emitted matmuls (the engine's wait-queue lookahead is
                # shallow, so emission order must match arrival order)
                return [
                    mk(c, half, nch, jj)
                    for half in range(2)
                    for nch in range(2)
                    for jj in range(DC)
                    for c in range(2)
                ]
            return [
                mk(c, half, nch, jj)
                for half in range(2)
                for c in range(2)
                for nch in range(2)
                for jj in range(DC)
            ]

        _BUFS = {"pp": 1, "st": 2, "acc": 1}

        # ------------- v projection thunks (natural layout, all heads) -------
        def vna_thunks(k):
            cell = {}

            def mk(jj):
                def emit():
                    if "ps" not in cell:
                        cell["ps"] = psum.tile(
                            [P, DY], F32, tag="st", bufs=2, name=f"vps{k}"
                        )
                    ps = cell["ps"]
                    nc.tensor.matmul(
                        ps,
                        xTf[:, jj, ts(k, P)],
                        wkvTf[:, jj, QKW : QKW + DY],
                        start=(jj == 0),
                        stop=(jj == DC - 1),
                    )
                    if jj == DC - 1:
                        nc.scalar.copy(out=vna[:, k, :], in_=ps)
                return emit

            return [mk(jj) for jj in range(DC)]

        # ------------- output projection thunks (n-tile i) -------------
        def oproj_thunks(i, tag, split=None):
            cell = {}

            def mk(eh, hh):
                def emit():
                    if "po" not in cell:
                        cell["po"] = psum.tile(
                            [P, D], F32, tag=tag, bufs=_BUFS[tag], name=f"po{i}"
                        )
                    po = cell["po"]
                    nc.tensor.matmul(
                        po[:, ts(eh, MM)],
                        yT[:, hh, ts(i, P)],
                        woTf[:, hh, eh * MM : (eh + 1) * MM],
                        start=(hh == 0),
                        stop=(hh == HPC - 1),
                    )
                    if eh == 1 and hh == HPC - 1:
                        ot = work.tile([P, D], F16, tag="so", bufs=3, name=f"ot{i}")
                        do_split = FLAGS["po_split"] if split is None else split
                        if do_split:
                            nc.scalar.copy(out=ot[:, 0:MM], in_=po[:, 0:MM])
                            nc.vector.tensor_copy(out=ot[:, MM:D], in_=po[:, MM:D])
                        else:
                            nc.scalar.copy(out=ot, in_=po)
                        nc.sync.dma_start(out=out.ap()[ts(i, P), :], in_=ot)
                return emit

            # hh-major: the hh==3 matmuls (which read the LAST head's yT,
            # written by a finisher deferred to k==1 of the consuming block)
            # sit at positions 6-7, so up to 6 thunks may be emitted at k==0
            return [mk(eh, hh) for hh in range(HPC) for eh in range(2)]

        # ------------- attention block for (head h, q-chunk qc) -------------
        def attention(h, qc, fillers, nfill, deferred=None, drain_prev=None,
                      fin_split=None):
            """One attention block.  Returns (finisher, drain) closures the
            CALLER emits inside the NEXT block: `drain` (the yacc PSUM->SBUF
            copy, split across ACT+DVE) right after the next block's first
            exp, `finisher` (den cross-partition reduce + normalize) at k==1.
            Emitting them here would head-of-line block the next block's exp
            stream / scores behind them."""
            hb = h % 2
            yacc = psum.tile([P, QW], F32, tag="acc", bufs=1, name=f"yacc{h}{qc}")
            dacc = None
            ets = []
            for k in range(KT):
                st = psum.tile([P, QW], F32, tag="st", bufs=2, name=f"st{h}{qc}{k}")
                for m in range(2):
                    nc.tensor.matmul(
                        st[:, ts(m, MM)],
                        kT[hb][:, ts(k, P)],
                        qT[hb][:, qc * QW + m * MM : qc * QW + (m + 1) * MM],
                        start=True,
                        stop=True,
                    )
                et = work.tile([P, QW], F16, tag="et", bufs=5, name=f"et{h}{qc}{k}")
                nc.scalar.activation(out=et, in_=st, func=EXP, scale=SCALE)
                if k == FLAGS["drain_at"] and drain_prev is not None:
                    drain_prev()
                if k == FLAGS["fin_at"] and deferred is not None:
                    deferred()
                # fillers run while the scalar engine works through exp
                for _ in range(nfill):
                    if fillers:
                        fillers.popleft()()
                for m in range(2):
                    nc.tensor.matmul(
                        yacc[:, ts(m, MM)],
                        vna[:, k, ts(h, HD)],
                        et[:, ts(m, MM)],
                        start=(k == 0),
                        stop=(k == KT - 1),
                    )
                # denominator: running fp16 sum of exp tiles on DVE
                if k == 0:
                    ets.append(et)
                elif k == 1:
                    dacc = work.tile([P, QW], F16, tag="dacc", bufs=2, name=f"da{h}{qc}{k}")
                    nc.vector.tensor_add(out=dacc, in0=ets[0], in1=et)
                else:
                    nd = work.tile([P, QW], F16, tag="dacc", bufs=2, name=f"da{h}{qc}{k}")
                    nc.vector.tensor_add(out=nd, in0=dacc, in1=et)
                    dacc = nd
            ysb = work.tile([P, QW], F32, tag="ysb", bufs=2, name=f"ysb{h}{qc}")
            dacc_f = dacc

            def drain():
                if FLAGS["drain_dve"]:
                    nc.vector.tensor_copy(out=ysb, in_=yacc)
                elif FLAGS["drain_split"]:
                    nc.scalar.copy(out=ysb[:, 0:MM], in_=yacc[:, 0:MM])
                    nc.vector.tensor_copy(out=ysb[:, MM:QW], in_=yacc[:, MM:QW])
                else:
                    nc.scalar.copy(out=ysb, in_=yacc)

            if not FLAGS["drain_defer"]:
                drain()
                drain = None

            def finisher():
                dmm = psum.tile([P, QW], F32, tag="st", bufs=2, name=f"dmm{h}{qc}")
                rc = work.tile([P, QW], F32, tag="rc", bufs=2, name=f"rc{h}{qc}")
                fsp = FLAGS["fin_split"] if fin_split is None else fin_split
                if fsp:
                    # per-half so each yT half is available ~1.1us sooner
                    for m in range(2):
                        nc.tensor.matmul(
                            dmm[:, ts(m, MM)], ones16, dacc_f[:, ts(m, MM)],
                            start=True, stop=True,
                        )
                        nc.vector.reciprocal_approx_fast(
                            out=rc[:, ts(m, MM)], in_=dmm[:, ts(m, MM)]
                        )
                        nc.vector.tensor_mul(
                            out=yT[:, h, qc * QW + m * MM : qc * QW + (m + 1) * MM],
                            in0=ysb[:, ts(m, MM)],
                            in1=rc[:, ts(m, MM)],
                        )
                else:
                    for m in range(2):
                        nc.tensor.matmul(
                            dmm[:, ts(m, MM)], ones16, dacc_f[:, ts(m, MM)],
                            start=True, stop=True,
                        )
                    nc.vector.reciprocal_approx_fast(out=rc, in_=dmm)
                    nc.vector.tensor_mul(out=yT[:, h, ts(qc, QW)], in0=ysb, in1=rc)

            return finisher, drain

        # ---------------- emission schedule ----------------
        from collections import deque

        # Kickoff, in consumption order: head-0 q/k weights + x slice 0
        # interleaved (so the PE can start while the rest lands), then x
        # slice 1 / the v block.
        step = FLAGS["kick_step"]
        for jp in range(0, DC, step):
            load_wkv_qk(0, jp, jp + step)
            load_x_slice(0, jp, jp + step)
        if FLAGS["v_first"]:
            load_wkv_v()
            load_x_slice(1)
        else:
            load_x_slice(1)
            load_wkv_v()

        # Prologue: head-0 q/k projection + ALL v n-tiles, in x-slice order.
        # The Tile scheduler reorders by data arrival; emission order only
        # sets priority.  DMA issues for the later slices and weights are
        # interleaved at fixed positions.
        h0 = proj_thunks(0, cell_tags=("pp", "acc"))
        # h0 layout: [q(half0) 16 | k(half0) 16 | q(half1) 16 | k(half1) 16]
        prologue = (
            h0[0:16] + h0[16:32]
            + [t for k in range(0, 8) for t in vna_thunks(k)]
            + h0[32:48] + h0[48:64]
            + [t for k in range(8, 16) for t in vna_thunks(k)]
        )
        emitted = 0
        for t in prologue:
            if emitted == 8:
                load_x_slice(2)
            elif emitted == 24:
                load_x_slice(3)
            elif emitted == 40:
                load_wkv_qk(1)
            elif emitted == 56:
                load_wo()
            t()
            emitted += 1

        fin = drn = None
        # Heads 1 and 2 project fully inside the previous head's two blocks.
        # With h3_late, head 3's half-0 cells fill head 2's blocks and its
        # half-1 cells fill block (3,0) itself -- qc0 attention only reads
        # q-half0, and the k-half1 tiles are first consumed at k==8, by
        # which point the cell (ordered first) has drained.
        h3 = proj_thunks(3, cell_tags=("pp",))
        if FLAGS["h3_late"]:
            plans = [
                (0, deque(proj_thunks(1, cell_tags=("pp",))), FLAGS["nf0"]),
                (1, deque(proj_thunks(2, cell_tags=("pp",))), FLAGS["nf1"]),
                (2, deque(h3[0:32]), FLAGS["nf2"]),
            ]
            late3 = deque(h3[48:64] + h3[32:48])  # k(half1) first, then q(half1)
        else:
            plans = [
                (0, deque(proj_thunks(1, cell_tags=("pp",))), FLAGS["nf0"]),
                (1, deque(proj_thunks(2, cell_tags=("pp",))), FLAGS["nf1"]),
                (2, deque(h3), FLAGS["nf2"]),
            ]
            late3 = deque()
        for h, fillers, nfill in plans:
            if h + 2 < HPC:
                load_wkv_qk(h + 2)
            fin, drn = attention(h, 0, fillers, nfill, deferred=fin, drain_prev=drn)
            fin, drn = attention(h, 1, fillers, nfill, deferred=fin, drain_prev=drn)
            while fillers:
                fillers.popleft()()
        fin, drn = attention(3, 0, late3, FLAGS["nf30"], deferred=fin, drain_prev=drn)
        while late3:
            late3.popleft()()
        # the first half of the output projection fills qc1 (it needs every
        # head's qc0 yT, complete once qc0's finisher has run); the rest
        # follows in the tail.
        op = deque()
        for i in range(NT // 2):
            op.extend(oproj_thunks(i, tag="pp"))
        fin, drn = attention(3, 1, op, FLAGS["nf31"], deferred=fin, drain_prev=drn,
                             fin_split=FLAGS["fin_split_last"])
        if drn is not None:
            drn()
        fin()
        while op:
            op.popleft()()
        tail_tags = (
            ["pp", "st", "acc", "st"] if FLAGS["tail4"] else ["pp", "st", "pp", "st"]
        )
        for i in range(NT // 2, NT):
            sp = FLAGS["po_split_tail"] and i >= NT - 4
            for t in oproj_thunks(i, tag=tail_tags[i % 4], split=sp):
                t()
    nc.finalize()
    return nc


def _get_nc():
    if "nc" not in _CACHE:
        _CACHE["nc"] = _build()
    return _CACHE["nc"]


def make_in_maps(x, Wkv, Wo):
    """Per-core input dicts for core = 2*b + g (host pre-transposes + casts)."""
    from ml_dtypes import bfloat16

    xts = [np.ascontiguousarray(x[b].T).astype(bfloat16) for b in range(B)]
    wkvts, wots = [], []
    for g in range(2):
        rows = np.concatenate(
            [
                Wkv[c * D + g * DY + h * HD : c * D + g * DY + (h + 1) * HD]
                for h in range(HPC)
                for c in range(2)
            ]
            + [Wkv[2 * D + g * DY : 2 * D + (g + 1) * DY]],
            axis=0,
        )  # [1536, 1024] rows: [q0 k0 q1 k1 q2 k2 q3 k3 | v(512)]
        wkvts.append(np.ascontiguousarray(rows.T).astype(bfloat16))
        wots.append(
            np.ascontiguousarray(Wo[:, g * DY : (g + 1) * DY].T).astype(np.float16)
        )
    in_maps = []
    for core in range(8):
        b, g = divmod(core, 2)
        in_maps.append({"xt": xts[b], "wkvt": wkvts[g], "wot": wots[g]})
    return in_maps


def gather_out(results, bo):
    out = np.empty((B, N, D), np.float32)
    for b in range(B):
        out[b] = np.asarray(results[2 * b]["out"], np.float32) + np.asarray(
            results[2 * b + 1]["out"], np.float32
        )
    out += bo.astype(np.float32)
    return out


def _numpy_reference(x, mask, Wkv, Wo, bo):
    """Exact fallback (used only if a nonzero additive mask is passed)."""
    x64 = x.astype(np.float64)
    qkv = x64 @ Wkv.T.astype(np.float64)
    q, k, v = np.split(qkv, 3, axis=-1)
    q = q.reshape(B, N, H, HD).transpose(0, 2, 1, 3)
    k = k.reshape(B, N, H, HD).transpose(0, 2, 1, 3)
    v = v.reshape(B, N, H, HD).transpose(0, 2, 1, 3)
    s = q @ k.transpose(0, 1, 3, 2) / np.sqrt(HD) + mask.astype(np.float64)
    s = s - s.max(axis=-1, keepdims=True)
    p = np.exp(s)
    p /= p.sum(axis=-1, keepdims=True)
    y = (p @ v).transpose(0, 2, 1, 3).reshape(B, N, D)
    return (y @ Wo.T.astype(np.float64) + bo.astype(np.float64)).astype(np.float32)


def kernel(x, mask, Wkv, Wo, bo):
    x = np.asarray(x, dtype=np.float32)
    mask = np.asarray(mask, dtype=np.float32)
    Wkv = np.asarray(Wkv, dtype=np.float32)
    Wo = np.asarray(Wo, dtype=np.float32)
    bo = np.asarray(bo, dtype=np.float32)
    if mask.size and np.abs(mask).max() != 0.0:
        return _numpy_reference(x, mask, Wkv, Wo, bo)

    from concourse.bass_utils import run_bass_kernel_spmd

    nc = _get_nc()
    res = run_bass_kernel_spmd(nc, make_in_maps(x, Wkv, Wo), core_ids=list(range(8)))
    return gather_out(res.results, bo)


if __name__ == "__main__":
    rng = np.random.default_rng(0)
    x = rng.standard_normal((B, N, D), dtype=np.float32)
    mask = np.zeros((N, N), np.float32)
    Wkv = (rng.standard_normal((3 * D, D), dtype=np.float32) / np.sqrt(D)).astype(np.float32)
    Wo = (rng.standard_normal((D, D), dtype=np.float32) / np.sqrt(D)).astype(np.float32)
    bo = np.zeros((D,), np.float32)
    got = kernel(x, mask, Wkv, Wo, bo)
    want = _numpy_reference(x, mask, Wkv, Wo, bo)
    err = np.linalg.norm(got - want) / np.linalg.norm(want)
    print("rel err:", err)


# revision 27
# speedup vs baseline: 2.5604x; 1.8462x over previous
"""Trainium2 Bass kernel for nn_Attention (B=4, N=2048, D=1024, H=8 heads).

Computes: qkv = x @ Wkv.T; q,k,v split into 8 heads of 128 dims;
y = softmax(q k^T / sqrt(128) + mask) v;  out = y @ Wo.T + bo.

Sharding (8 NeuronCores): core (b, g) = batch b in 0..3, head-group g in 0..1
(4 heads each).  Each core computes its 4 heads' attention and a partial
output projection; the host sums the two head-group partials per batch and
adds bo.

The additive mask is skipped on device: the problem spec fills it with zeros
(exp(s + 0) == exp(s)).  If a nonzero mask is ever passed, kernel() falls back
to an exact numpy implementation.

Device-side design (v3):
 - All matmul operands are 16-bit (full PE rate): x/Wkv in bf16, q/k/v/
   exp-tiles/Wo in fp16.  Accumulation stays fp32 in PSUM.
 - The host pre-transposes AND pre-casts x, Wkv and Wo slices per core,
   so the device does NO transposes and no casts.  wkvt is laid out
   [q0 k0 | q1 k1 | q2 k2 | q3 k3 | v(all 4 heads, 512)] so that
   (a) q/k project per head into the transposed [d, n] layout the
   scores matmul wants, and (b) v projects DIRECTLY into its natural
   [n, d] layout (lhsT = x tile, rhs = the 512-wide v weight block),
   which removes all 64 PE-transposes and their DVE drains from v2.
 - Scores are computed TRANSPOSED (sT[k, q] = kT_tile.T @ qT) so softmax
   needs no p transposes before the PV matmul.
 - exp runs on the scalar engine reading scores from PSUM with the
   1/sqrt(128) scale fused in, writing fp16 tiles to SBUF.
 - The softmax denominator is a running fp16 elementwise sum of the 16
   exp tiles on DVE, finished by a single ones-matmul for the
   cross-partition reduction.
 - Emission: prologue = head-0 q/k projection (j-interleaved to match the
   batched-DMA arrival order) + ALL v n-tiles.  Heads 1-3's q/k
   projections and the first half of the output projection are
   interleaved into the attention blocks as single-matmul fillers so the
   PE never drains while the scalar engine works through the exp stream;
   head-3's half-1 cells fill its own qc0 block.  Each block's yacc
   drain/finisher are deferred into the next block (k==0/k==1).  Input
   DMAs are batched (one instruction per logical block) -- each dma_start
   costs ~500ns of issue + a 900ns completion semaphore.
 - PSUM budget (8 banks): scores "st" 2x[128,1024]f32 (4; also time-shares
   with the v-projection accumulator and the den ones-matmul), yacc "acc"
   (2; time-shares with head-0 k cells), proj/oproj "pp" (2).
 - Schedule knobs live in FLAGS (defaults are the CoreSim-tuned winners:
   234.7us predicted vs 248.2us for the v2 schedule).
"""

import numpy as np

B, N, D, H = 4, 2048, 1024, 8
HD = D // H          # 128 head dim
HPC = H // 2         # 4 heads per core
DY = HPC * HD        # 512 local y dims per core
P = 128
NT = N // P          # 16 n-tiles
DC = D // P          # 8 d-chunks
KT = N // P          # 16 k-tiles
QC = 2               # q chunks per head
QW = N // QC         # 1024 q width
MM = 512             # max fp32 moving free dim
NS = 4               # x load n-slices
SW = N // NS         # 512 slice width
QKW = HPC * 2 * HD   # 1024 q/k columns in wkvt
SCALE = float(1.0 / np.sqrt(HD))

# Schedule knobs (resolved at _build time; the shipped defaults are the
# sim-tuned winners).
FLAGS = dict(
    kick_step=4,      # j-granularity of the kickoff qk/x interleave
    v_first=False,    # v-block DMA before x slice 1
    drain_split=True,   # yacc drain split ACT/DVE vs single scalar copy
    drain_defer=True,   # defer yacc drain into the next block (at k==0)
    fin_split=False,  # half-split finisher (dmm/recip/mul per 512)
    h3_late=True,     # head-3 half-1 projection inside block (3,0)
    po_split=False,   # oproj drain split ACT/DVE
    tail4=True,       # tail oproj over 4 psum slots instead of 2
    nf0=2, nf1=1, nf2=1, nf30=3, nf31=5,  # per-block filler counts
    h0_jzip=True,      # j-interleave head-0 q/k cells in the prologue
    fin_split_last=True,   # half-split only the final finisher
    po_split_tail=False,   # split drains for the last four oproj tiles
    drain_dve=False,  # yacc drain entirely on DVE
    drain_at=0,       # k at which the previous block's yacc drain is emitted
    fin_at=1,         # k at which the previous block's finisher is emitted
    out_batch=1,      # output n-tiles per DMA (1 = DMA per tile)
)

_CACHE = {}


def _build():
    from contextlib import ExitStack

    import concourse.bacc as bacc
    import concourse.bass as bass
    import concourse.mybir as mybir
    from concourse.tile import TileContext

    ts = bass.ts
    F32 = mybir.dt.float32
    F16 = mybir.dt.float16
    BF16 = mybir.dt.bfloat16
    EXP = mybir.ActivationFunctionType.Exp

    nc = bacc.Bacc("TRN2", target_bir_lowering=False, debug=False)
    # Host-pre-transposed, host-pre-cast inputs (see make_in_maps):
    #   xt[d, n]                    = x[b][n, d]                      (bf16)
    #   wkvt[d, h*256 + 128c + i]   = W{q,k}[g*DY + h*HD + i, d]      (bf16)
    #   wkvt[d, 1024 + j]           = Wv[g*DY + j, d]                 (bf16)
    #   wot[p, h*D + e]             = Wo[e, g*DY + h*HD + p]          (fp16)
    xt = nc.dram_tensor("xt", [D, N], BF16, kind="ExternalInput")
    wkvt = nc.dram_tensor("wkvt", [D, 3 * DY], BF16, kind="ExternalInput")
    wot = nc.dram_tensor("wot", [DY, D], F16, kind="ExternalInput")
    # fp16 output halves the output DMA; the host sums the two head-group
    # partials in fp32.
    out = nc.dram_tensor("out", [N, D], F16, kind="ExternalOutput")

    with TileContext(nc) as tc, ExitStack() as top:
        consts = top.enter_context(tc.tile_pool(name="consts", bufs=1))
        ones32 = consts.tile([P, P], F32, tag="ones32")
        nc.vector.memset(ones32, 1.0)
        ones16 = consts.tile([P, P], F16, tag="ones16")
        nc.vector.tensor_copy(out=ones16, in_=ones32)

        persist = top.enter_context(tc.tile_pool(name="persist", bufs=1))
        xTf = persist.tile([P, DC, N], BF16, tag="xTf")
        wkvTf = persist.tile([P, DC, 3 * DY], BF16, tag="wkvTf")
        woTf = persist.tile([P, HPC, D], F16, tag="woTf")
        # qT/kT are double-buffered on head parity so head h+1's projection
        # (interleaved into head h's attention) never overwrites tiles
        # attention is still reading.  v (vna) is shared by all heads and
        # computed once in the prologue, so it needs no parity buffer.
        qT = [persist.tile([P, N], F16, tag=f"qT{i}", name=f"qT{i}") for i in range(2)]
        kT = [persist.tile([P, N], F16, tag=f"kT{i}", name=f"kT{i}") for i in range(2)]
        vna = persist.tile([P, NT, DY], F16, tag="vna")
        yT = persist.tile([P, HPC, N], F16, tag="yT")
        obuf = (
            persist.tile([P, NT, D], F16, tag="obuf", name="obuf")
            if FLAGS["out_batch"] > 1
            else None
        )

        work = top.enter_context(tc.tile_pool(name="work", bufs=1))
        psum = top.enter_context(tc.tile_pool(name="ps", bufs=1, space="PSUM"))

        # -------- loaders (batched DMA: one instruction per logical block;
        # each dma_start costs ~500ns of SP issue + a completion semaphore,
        # so fewer/bigger transfers start the pipeline much sooner) --------
        def load_x_slice(ns, j0=0, j1=DC):
            nc.sync.dma_start(
                out=xTf[:, j0:j1, ts(ns, SW)],
                in_=xt.ap()[j0 * P : j1 * P, ts(ns, SW)].rearrange(
                    "(j p) n -> p j n", p=P
                ),
            )

        def load_wkv_qk(h, j0=0, j1=DC):
            nc.sync.dma_start(
                out=wkvTf[:, j0:j1, ts(h, 2 * HD)],
                in_=wkvt.ap()[j0 * P : j1 * P, ts(h, 2 * HD)].rearrange(
                    "(j p) c -> p j c", p=P
                ),
            )

        def load_wkv_v():
            nc.sync.dma_start(
                out=wkvTf[:, :, QKW : QKW + DY],
                in_=wkvt.ap()[:, QKW : QKW + DY].rearrange("(j p) c -> p j c", p=P),
            )

        def load_wo():
            nc.sync.dma_start(
                out=woTf,
                in_=wot.ap().rearrange("(h p) e -> p h e", p=P),
            )

        # ------------- q/k projection thunks for head h -------------
        # Each thunk emits one PE instruction (plus the trailing DVE drain),
        # so attention blocks can interleave them as fillers.  A cell is one
        # (c, half) pair: a [128, 1024] PSUM accumulation over the 8 d-chunks
        # x 2 n-halves, drained to qT/kT once complete.
        def proj_thunks(h, cell_tags):
            hb = h % 2
            cells = {}
            tag_of = {}
            for idx, key in enumerate((c, half) for half in range(2) for c in range(2)):
                tag_of[key] = cell_tags[idx % len(cell_tags)]
            dests = {0: qT[hb], 1: kT[hb]}

            def mk(c, half, nch, jj):
                def emit():
                    key = (c, half)
                    if key not in cells:
                        cells[key] = psum.tile(
                            [P, QW], F32, tag=tag_of[key], bufs=_BUFS[tag_of[key]],
                            name=f"pp{h}{c}{half}",
                        )
                    pp = cells[key]
                    col0 = h * 2 * HD + c * HD
                    nc.tensor.matmul(
                        pp[:, ts(nch, MM)],
                        wkvTf[:, jj, col0 : col0 + HD],
                        xTf[:, jj, half * QW + nch * MM : half * QW + (nch + 1) * MM],
                        start=(jj == 0),
                        stop=(jj == DC - 1),
                    )
                    if jj == DC - 1 and nch == 1:
                        nc.vector.tensor_copy(out=dests[c][:, ts(half, QW)], in_=pp)
                return emit

            if FLAGS["h0_jzip"] and cell_tags != ("pp",):
                # head-0 prologue: j-interleave the q and k cells so every
                # arriving x/wkv j-chunk immediately unlocks the next two
                # emitted matmuls (the engine's wait-queue lookahead is
                # shallow, so emission order must match arrival order)
                return [
                    mk(c, half, nch, jj)
                    for half in range(2)
                    for nch in range(2)
                    for jj in range(DC)
                    for c in range(2)
                ]
            return [
                mk(c, half, nch, jj)
                for half in range(2)
                for c in range(2)
                for nch in range(2)
                for jj in range(DC)
            ]

        _BUFS = {"pp": 1, "st": 2, "acc": 1}

        # ------------- v projection thunks (natural layout, all heads) -------
        def vna_thunks(k):
            cell = {}

            def mk(jj):
                def emit():
                    if "ps" not in cell:
                        cell["ps"] = psum.tile(
                            [P, DY], F32, tag="st", bufs=2, name=f"vps{k}"
                        )
                    ps = cell["ps"]
                    nc.tensor.matmul(
                        ps,
                        xTf[:, jj, ts(k, P)],
                        wkvTf[:, jj, QKW : QKW + DY],
                        start=(jj == 0),
                        stop=(jj == DC - 1),
                    )
                    if jj == DC - 1:
                        nc.scalar.copy(out=vna[:, k, :], in_=ps)
                return emit

            return [mk(jj) for jj in range(DC)]

        # ------------- output projection thunks (n-tile i) -------------
        def oproj_thunks(i, tag, split=None):
            cell = {}

            def mk(eh, hh):
                def emit():
                    if "po" not in cell:
                        cell["po"] = psum.tile(
                            [P, D], F32, tag=tag, bufs=_BUFS[tag], name=f"po{i}"
                        )
                    po = cell["po"]
                    nc.tensor.matmul(
                        po[:, ts(eh, MM)],
                        yT[:, hh, ts(i, P)],
                        woTf[:, hh, eh * MM : (eh + 1) * MM],
                        start=(hh == 0),
                        stop=(hh == HPC - 1),
                    )
                    if eh == 1 and hh == HPC - 1:
                        gb = FLAGS["out_batch"]
                        if gb > 1:
                            ot = obuf[:, i, :]
                        else:
                            ot = work.tile(
                                [P, D], F16, tag="so", bufs=3, name=f"ot{i}"
                            )
                        do_split = FLAGS["po_split"] if split is None else split
                        if do_split:
                            nc.scalar.copy(out=ot[:, 0:MM], in_=po[:, 0:MM])
                            nc.vector.tensor_copy(out=ot[:, MM:D], in_=po[:, MM:D])
                        else:
                            nc.scalar.copy(out=ot, in_=po)
                        if gb == 1:
                            nc.sync.dma_start(out=out.ap()[ts(i, P), :], in_=ot)
                        elif i % gb == gb - 1:
                            i0 = i - gb + 1
                            nc.sync.dma_start(
                                out=out.ap()[i0 * P : (i + 1) * P, :].rearrange(
                                    "(t p) e -> p t e", p=P
                                ),
                                in_=obuf[:, i0 : i + 1, :],
                            )
                return emit

            # hh-major: the hh==3 matmuls (which read the LAST head's yT,
            # written by a finisher deferred to k==1 of the consuming block)
            # sit at positions 6-7, so up to 6 thunks may be emitted at k==0
            return [mk(eh, hh) for hh in range(HPC) for eh in range(2)]

        # ------------- attention block for (head h, q-chunk qc) -------------
        def attention(h, qc, fillers, nfill, deferred=None, drain_prev=None,
                      fin_split=None):
            """One attention block.  Returns (finisher, drain) closures the
            CALLER emits inside the NEXT block: `drain` (the yacc PSUM->SBUF
            copy, split across ACT+DVE) right after the next block's first
            exp, `finisher` (den cross-partition reduce + normalize) at k==1.
            Emitting them here would head-of-line block the next block's exp
            stream / scores behind them."""
            hb = h % 2
            yacc = psum.tile([P, QW], F32, tag="acc", bufs=1, name=f"yacc{h}{qc}")
            dacc = None
            ets = []
            for k in range(KT):
                st = psum.tile([P, QW], F32, tag="st", bufs=2, name=f"st{h}{qc}{k}")
                for m in range(2):
                    nc.tensor.matmul(
                        st[:, ts(m, MM)],
                        kT[hb][:, ts(k, P)],
                        qT[hb][:, qc * QW + m * MM : qc * QW + (m + 1) * MM],
                        start=True,
                        stop=True,
                    )
                et = work.tile([P, QW], F16, tag="et", bufs=5, name=f"et{h}{qc}{k}")
                nc.scalar.activation(out=et, in_=st, func=EXP, scale=SCALE)
                if k == FLAGS["drain_at"] and drain_prev is not None:
                    drain_prev()
                if k == FLAGS["fin_at"] and deferred is not None:
                    deferred()
                # fillers run while the scalar engine works through exp
                for _ in range(nfill):
                    if fillers:
                        fillers.popleft()()
                for m in range(2):
                    nc.tensor.matmul(
                        yacc[:, ts(m, MM)],
                        vna[:, k, ts(h, HD)],
                        et[:, ts(m, MM)],
                        start=(k == 0),
                        stop=(k == KT - 1),
                    )
                # denominator: running fp16 sum of exp tiles on DVE
                if k == 0:
                    ets.append(et)
                elif k == 1:
                    dacc = work.tile([P, QW], F16, tag="dacc", bufs=2, name=f"da{h}{qc}{k}")
                    nc.vector.tensor_add(out=dacc, in0=ets[0], in1=et)
                else:
                    nd = work.tile([P, QW], F16, tag="dacc", bufs=2, name=f"da{h}{qc}{k}")
                    nc.vector.tensor_add(out=nd, in0=dacc, in1=et)
                    dacc = nd
            ysb = work.tile([P, QW], F32, tag="ysb", bufs=2, name=f"ysb{h}{qc}")
            dacc_f = dacc

            def drain():
                if FLAGS["drain_dve"]:
                    nc.vector.tensor_copy(out=ysb, in_=yacc)
                elif FLAGS["drain_split"]:
                    nc.scalar.copy(out=ysb[:, 0:MM], in_=yacc[:, 0:MM])
                    nc.vector.tensor_copy(out=ysb[:, MM:QW], in_=yacc[:, MM:QW])
                else:
                    nc.scalar.copy(out=ysb, in_=yacc)

            if not FLAGS["drain_defer"]:
                drain()
                drain = None

            def finisher():
                dmm = psum.tile([P, QW], F32, tag="st", bufs=2, name=f"dmm{h}{qc}")
                rc = work.tile([P, QW], F32, tag="rc", bufs=2, name=f"rc{h}{qc}")
                fsp = FLAGS["fin_split"] if fin_split is None else fin_split
                if fsp:
                    # per-half so each yT half is available ~1.1us sooner
                    for m in range(2):
                        nc.tensor.matmul(
                            dmm[:, ts(m, MM)], ones16, dacc_f[:, ts(m, MM)],
                            start=True, stop=True,
                        )
                        nc.vector.reciprocal_approx_fast(
                            out=rc[:, ts(m, MM)], in_=dmm[:, ts(m, MM)]
                        )
                        nc.vector.tensor_mul(
                            out=yT[:, h, qc * QW + m * MM : qc * QW + (m + 1) * MM],
                            in0=ysb[:, ts(m, MM)],
                            in1=rc[:, ts(m, MM)],
                        )
                else:
                    for m in range(2):
                        nc.tensor.matmul(
                            dmm[:, ts(m, MM)], ones16, dacc_f[:, ts(m, MM)],
                            start=True, stop=True,
                        )
                    nc.vector.reciprocal_approx_fast(out=rc, in_=dmm)
                    nc.vector.tensor_mul(out=yT[:, h, ts(qc, QW)], in0=ysb, in1=rc)

            return finisher, drain

        # ---------------- emission schedule ----------------
        from collections import deque

        # Kickoff, in consumption order: head-0 q/k weights + x slice 0
        # interleaved (so the PE can start while the rest lands), then x
        # slice 1 / the v block.
        step = FLAGS["kick_step"]
        for jp in range(0, DC, step):
            load_wkv_qk(0, jp, jp + step)
            load_x_slice(0, jp, jp + step)
        if FLAGS["v_first"]:
            load_wkv_v()
            load_x_slice(1)
        else:
            load_x_slice(1)
            load_wkv_v()

        # Prologue: head-0 q/k projection + ALL v n-tiles, in x-slice order.
        # The Tile scheduler reorders by data arrival; emission order only
        # sets priority.  DMA issues for the later slices and weights are
        # interleaved at fixed positions.
        h0 = proj_thunks(0, cell_tags=("pp", "acc"))
        # h0 layout: [q(half0) 16 | k(half0) 16 | q(half1) 16 | k(half1) 16]
        prologue = (
            h0[0:16] + h0[16:32]
            + [t for k in range(0, 8) for t in vna_thunks(k)]
            + h0[32:48] + h0[48:64]
            + [t for k in range(8, 16) for t in vna_thunks(k)]
        )
        emitted = 0
        for t in prologue:
            if emitted == 8:
                load_x_slice(2)
            elif emitted == 24:
                load_x_slice(3)
            elif emitted == 40:
                load_wkv_qk(1)
            elif emitted == 56:
                load_wo()
            t()
            emitted += 1

        fin = drn = None
        # Heads 1 and 2 project fully inside the previous head's two blocks.
        # With h3_late, head 3's half-0 cells fill head 2's blocks and its
        # half-1 cells fill block (3,0) itself -- qc0 attention only reads
        # q-half0, and the k-half1 tiles are first consumed at k==8, by
        # which point the cell (ordered first) has drained.
        h3 = proj_thunks(3, cell_tags=("pp",))
        if FLAGS["h3_late"]:
            plans = [
                (0, deque(proj_thunks(1, cell_tags=("pp",))), FLAGS["nf0"]),
                (1, deque(proj_thunks(2, cell_tags=("pp",))), FLAGS["nf1"]),
                (2, deque(h3[0:32]), FLAGS["nf2"]),
            ]
            late3 = deque(h3[48:64] + h3[32:48])  # k(half1) first, then q(half1)
        else:
            plans = [
                (0, deque(proj_thunks(1, cell_tags=("pp",))), FLAGS["nf0"]),
                (1, deque(proj_thunks(2, cell_tags=("pp",))), FLAGS["nf1"]),
                (2, deque(h3), FLAGS["nf2"]),
            ]
            late3 = deque()
        for h, fillers, nfill in plans:
            if h + 2 < HPC:
                load_wkv_qk(h + 2)
            fin, drn = attention(h, 0, fillers, nfill, deferred=fin, drain_prev=drn)
            fin, drn = attention(h, 1, fillers, nfill, deferred=fin, drain_prev=drn)
            while fillers:
                fillers.popleft()()
        fin, drn = attention(3, 0, late3, FLAGS["nf30"], deferred=fin, drain_prev=drn)
        while late3:
            late3.popleft()()
        # the first half of the output projection fills qc1 (it needs every
        # head's qc0 yT, complete once qc0's finisher has run); the rest
        # follows in the tail.
        op = deque()
        for i in range(NT // 2):
            op.extend(oproj_thunks(i, tag="pp"))
        fin, drn = attention(3, 1, op, FLAGS["nf31"], deferred=fin, drain_prev=drn,
                             fin_split=FLAGS["fin_split_last"])
        if drn is not None:
            drn()
        fin()
        while op:
            op.popleft()()
        tail_tags = (
            ["pp", "st", "acc", "st"] if FLAGS["tail4"] else ["pp", "st", "pp", "st"]
        )
        for i in range(NT // 2, NT):
            sp = FLAGS["po_split_tail"] and i >= NT - 4
            for t in oproj_thunks(i, tag=tail_tags[i % 4], split=sp):
                t()
    nc.finalize()
    return nc


def _get_nc():
    if "nc" not in _CACHE:
        _CACHE["nc"] = _build()
    return _CACHE["nc"]


def make_in_maps(x, Wkv, Wo):
    """Per-core input dicts for core = 2*b + g (host pre-transposes + casts)."""
    from ml_dtypes import bfloat16

    xts = [np.ascontiguousarray(x[b].T).astype(bfloat16) for b in range(B)]
    wkvts, wots = [], []
    for g in range(2):
        rows = np.concatenate(
            [
                Wkv[c * D + g * DY + h * HD : c * D + g * DY + (h + 1) * HD]
                for h in range(HPC)
                for c in range(2)
            ]
            + [Wkv[2 * D + g * DY : 2 * D + (g + 1) * DY]],
            axis=0,
        )  # [1536, 1024] rows: [q0 k0 q1 k1 q2 k2 q3 k3 | v(512)]
        wkvts.append(np.ascontiguousarray(rows.T).astype(bfloat16))
        wots.append(
            np.ascontiguousarray(Wo[:, g * DY : (g + 1) * DY].T).astype(np.float16)
        )
    in_maps = []
    for core in range(8):
        b, g = divmod(core, 2)
        in_maps.append({"xt": xts[b], "wkvt": wkvts[g], "wot": wots[g]})
    return in_maps


def gather_out(results, bo):
    out = np.empty((B, N, D), np.float32)
    for b in range(B):
        out[b] = np.asarray(results[2 * b]["out"], np.float32) + np.asarray(
            results[2 * b + 1]["out"], np.float32
        )
    out += bo.astype(np.float32)
    return out


def _numpy_reference(x, mask, Wkv, Wo, bo):
    """Exact fallback (used only if a nonzero additive mask is passed)."""
    x64 = x.astype(np.float64)
    qkv = x64 @ Wkv.T.astype(np.float64)
    q, k, v = np.split(qkv, 3, axis=-1)
    q = q.reshape(B, N, H, HD).transpose(0, 2, 1, 3)
    k = k.reshape(B, N, H, HD).transpose(0, 2, 1, 3)
    v = v.reshape(B, N, H, HD).transpose(0, 2, 1, 3)
    s = q @ k.transpose(0, 1, 3, 2) / np.sqrt(HD) + mask.astype(np.float64)
    s = s - s.max(axis=-1, keepdims=True)
    p = np.exp(s)
    p /= p.sum(axis=-1, keepdims=True)
    y = (p @ v).transpose(0, 2, 1, 3).reshape(B, N, D)
    return (y @ Wo.T.astype(np.float64) + bo.astype(np.float64)).astype(np.float32)


def kernel(x, mask, Wkv, Wo, bo):
    x = np.asarray(x, dtype=np.float32)
    mask = np.asarray(mask, dtype=np.float32)
    Wkv = np.asarray(Wkv, dtype=np.float32)
    Wo = np.asarray(Wo, dtype=np.float32)
    bo = np.asarray(bo, dtype=np.float32)
    if mask.size and np.abs(mask).max() != 0.0:
        return _numpy_reference(x, mask, Wkv, Wo, bo)

    from concourse.bass_utils import run_bass_kernel_spmd

    nc = _get_nc()
    res = run_bass_kernel_spmd(nc, make_in_maps(x, Wkv, Wo), core_ids=list(range(8)))
    return gather_out(res.results, bo)


if __name__ == "__main__":
    rng = np.random.default_rng(0)
    x = rng.standard_normal((B, N, D), dtype=np.float32)
    mask = np.zeros((N, N), np.float32)
    Wkv = (rng.standard_normal((3 * D, D), dtype=np.float32) / np.sqrt(D)).astype(np.float32)
    Wo = (rng.standard_normal((D, D), dtype=np.float32) / np.sqrt(D)).astype(np.float32)
    bo = np.zeros((D,), np.float32)
    got = kernel(x, mask, Wkv, Wo, bo)
    want = _numpy_reference(x, mask, Wkv, Wo, bo)
    err = np.linalg.norm(got - want) / np.linalg.norm(want)
    print("rel err:", err)
